# revision 29
# baseline (speedup 1.0000x reference)
"""COMPASSNet MoE-routing kernel for 8 TRN2 NeuronCores.

Problem: B=262144 samples of D=32 features with NaNs at 0/1/2 positions;
each of P=529 NaN patterns owns a tiny MLP (32 -> 4 -> 1, tanh/sigmoid).
y[b] = sigmoid(W2[p].tanh(x0[b] @ W1[p] + b1[p]) + b2[p]), p = pattern id.

Sharding strategy (host side, part of constructing per-core shards):
samples are grouped by pattern (stable sort of pattern_ids), patterns are
greedy bin-packed across the 8 cores, and each pattern group is padded to
a multiple of 128 sample slots.  All per-pattern parameters are folded
into dense per-tile operand streams so the device kernel is a fully
static, branch-free pipeline at the memory roofline.

Device kernel (SPMD, identical program on all 8 cores):
  - A "tile" = 512 sample slots packed 4-per-PE-column: the stationary
    matmul operand X4[t] is (K=128 = 4 slots x 32 features, M=128
    columns).  The moving operand is a (128, 20) block-diagonal weight
    matrix (slot s rows 32s..32s+31, cols 5s..5s+4 hold W1[pattern of
    slot s]).  One PE matmul per 512 samples -> h_pre in PSUM with
    samples on partitions.
  - b1 / W2 / b2 are applied with rank-1 (K=1 ones-column) matmuls that
    broadcast host-prepared per-tile rows across all 128 partitions.
  - tanh on ACT, H*W2 multiply + segment-sum(5) on DVE, sigmoid (bf16)
    into a staging buffer, sliced DMAs out.  Host unscrambles order.

DMA schedule (the perf-critical part): x and the per-tile weights are
interleaved host-side into one [128, T, 148] stream held in a single
SBUF tile, fetched in ~6 chunks alternating between the two HWDGE rings
(SP / ACT) so data movement, not descriptor issue, gates the stream;
bias/W2 rows ride the SWDGE (gpsimd) queue; outputs go back on the SP
ring in three slices.  First/last chunks are small to shorten pipeline
fill and the end-of-kernel receipt tail, and the [W2|b2] broadcast is
computed once into PSUM and copied to fp16 SBUF so the whole DVE
post-chain runs at 16-bit rate with no per-chunk PSUM-buffer hazards.
"""

import ml_dtypes
import numpy as np

import concourse.bass as bass
import concourse.tile as tile
from concourse import mybir
from concourse.bass_utils import run_bass_kernel_spmd

F32 = mybir.dt.float32
BF16 = mybir.dt.bfloat16
FP16 = mybir.dt.float16
MM_DT = BF16          # dtype of the big matmul operand streams
MM_NP = ml_dtypes.bfloat16 if MM_DT == BF16 else np.float32

B = 262144
D = 32
P = 529
H = 4
H5 = 5          # hidden + ones column (b2 folded into W2)
N_CORES = 8
SLOT = 128          # pattern groups padded to multiples of this
TILE = 512          # samples per PE stationary tile (4 slots x 128 cols)


# ----------------------------------------------------------------- host pack
def _pack(x, pattern_ids, W1, b1, W2, b2):
    """Build per-core device operand streams. Returns (T, in_maps, scatter)."""
    pid = np.asarray(pattern_ids).astype(np.int64).ravel()
    x = np.asarray(x, dtype=np.float32)
    W1 = np.asarray(W1, dtype=np.float32)
    b1 = np.asarray(b1, dtype=np.float32)
    W2 = np.asarray(W2, dtype=np.float32)
    b2 = np.asarray(b2, dtype=np.float32)

    order = np.argsort(pid, kind="stable")
    counts = np.bincount(pid, minlength=P)
    starts = np.zeros(P + 1, np.int64)
    np.cumsum(counts, out=starts[1:])

    # greedy bin-pack patterns over cores by 128-slot units
    units = (counts + SLOT - 1) // SLOT          # slot units per pattern
    pat_order = np.argsort(-counts, kind="stable")
    core_units = np.zeros(N_CORES, np.int64)
    core_pats = [[] for _ in range(N_CORES)]
    for p in pat_order:
        c = int(np.argmin(core_units))
        core_pats[c].append(int(p))
        core_units[c] += units[p]
    T = int((core_units.max() * SLOT + TILE - 1) // TILE)

    S = T * TILE
    in_maps = []
    scatter = []                                  # (orig_indices, packed_pos)
    for c in range(N_CORES):
        idx = np.full(S, -1, np.int64)            # packed slot -> orig sample
        slot_pat = np.zeros(T * 4, np.int64)      # 128-slot block -> pattern
        pos = 0
        for p in core_pats[c]:
            n = int(counts[p])
            if n:
                idx[pos:pos + n] = order[starts[p]:starts[p] + n]
            nblk = (n + SLOT - 1) // SLOT
            slot_pat[pos // SLOT: pos // SLOT + nblk] = p
            pos += nblk * SLOT
        valid = idx >= 0
        x0 = np.zeros((S, D), np.float32)
        xv = x[idx[valid]]
        np.nan_to_num(xv, copy=False)
        x0[valid] = xv

        # X4r[p=32s+d, t, m] = x0[t*512 + s*128 + m, d]
        X4 = x0.reshape(T, 4, SLOT, D).transpose(0, 1, 3, 2).reshape(T, 128, 128)
        X4r = np.ascontiguousarray(X4.transpose(1, 0, 2)).astype(MM_NP)

        sp = slot_pat.reshape(T, 4)
        # block-diagonal W1 per tile, hidden extended to H5=5: the 5th
        # column is 0 in W1, 20.0 in b1 (tanh(20) == 1.0f), and b2 in W2 —
        # so layer 2's bias rides the weighted reduce for free.
        W1e = np.zeros((P, D, H5), np.float32)
        W1e[:, :, :H] = W1
        b1e = np.full((P, H5), 20.0, np.float32)
        b1e[:, :H] = b1
        W2e = np.zeros((P, H5), np.float32)
        W2e[:, :H] = W2
        W2e[:, H] = b2
        WB = np.zeros((T, 4, D, 4, H5), np.float32)
        s4 = np.arange(4)
        WB[:, s4, :, s4, :] = W1e[sp].transpose(1, 0, 2, 3)
        WBr = np.ascontiguousarray(
            WB.reshape(T, 128, 4 * H5).transpose(1, 0, 2)).astype(MM_NP)

        # bias row then W2 row, concatenated -> one DMA (fp16: feeds the
        # rank-1 bias matmul and the on-chip W2 partition-broadcast)
        BW = np.concatenate(
            [b1e[sp].reshape(-1), W2e[sp].reshape(-1)])[None, :]

        in_maps.append({
            "xw": np.ascontiguousarray(np.concatenate([X4r, WBr], axis=2)),
            "bw": np.ascontiguousarray(BW).astype(np.float16),
        })
        scatter.append((idx, valid))
    return T, in_maps, scatter


# ------------------------------------------------------------- device build
def _split_excess_waits(nc, cap=1):
    """walrus here rejects >1 sync wait per instruction; move extras onto
    same-engine NoOps placed immediately before the owner."""
    f = nc.m.functions[0]
    for bb in list(f.blocks):
        out, changed = [], False
        for inst in bb.instructions:
            si = inst.sync_info
            waits = list(si.on_wait) if si is not None else []
            if len(waits) > cap:
                for w in waits[:-cap]:
                    out.append(mybir.InstNoOp(
                        name=nc.get_next_instruction_name(),
                        sync_info=mybir.SyncInfo(on_wait=[w], on_update=[]),
                        bass_nofuse=True,
                        engine=inst.engine,
                    ))
                si.on_wait = waits[-cap:]
                changed = True
            out.append(inst)
        if changed:
            bb.instructions = out


def _chunk_plan(T):
    """x-stream chunk sizes: small first (early compute start), ~16-tile
    mids (data-gated streaming), small last (short receipt tail)."""
    if T <= 12:
        return [T]
    tail = [6, 3]
    rem = T - sum(tail)
    n = max(1, (rem + 15) // 16)
    base, extra = divmod(rem, n)
    mids = [base + (1 if i < extra else 0) for i in range(n)]
    return [m for m in mids if m] + tail


def _build(T, split_waits=True):
    nc = bass.Bass("TRN2", target_bir_lowering=False, debug=False)
    xw = nc.declare_dram_parameter("xw", [128, T, 148], MM_DT, isOutput=False)
    bw = nc.declare_dram_parameter("bw", [1, 2 * T * 4 * H5], FP16, isOutput=False)
    y = nc.declare_dram_parameter("y", [128, T * 4], MM_DT, isOutput=True)

    dchunks = _chunk_plan(T)
    dbounds = [0]
    for c in dchunks:
        dbounds.append(dbounds[-1] + c)
    # compute chunks: the last two (small) DMA chunks share one PSUM tile
    # and one tanh, shortening the end-of-stream serial chain
    if len(dchunks) >= 3:
        chunks = dchunks[:-2] + [dchunks[-2] + dchunks[-1]]
    else:
        chunks = list(dchunks)
    C = len(chunks)
    bounds = [0]
    for c in chunks:
        bounds.append(bounds[-1] + c)
    TW = T * 4 * H5

    # post-processing batches (tile ranges), each ending in one output DMA;
    # the last batch is the small merged tail chunk so the chain is short
    if C >= 3:
        # cut batch2 at a mid-stream chunk boundary so its mul/reduce runs
        # while data still streams; only ONE TT/TR pair remains on the tail
        cuts = sorted({bounds[min(2, C - 1)], bounds[min(3, C - 1)],
                       bounds[min(4, C - 1)], T})
        batches = [(a, b) for a, b in zip([0] + cuts, cuts) if b > a]
    else:
        batches = [(0, T)]

    with tile.TileContext(nc) as tc:
        with (
            tc.tile_pool(name="consts", bufs=1) as consts,
            tc.tile_pool(name="ps1", bufs=5, space="PSUM") as ps1p,
            tc.tile_pool(name="ps2", bufs=1, space="PSUM") as ps2p,
        ):
            ones = consts.tile([1, 128], FP16)
            nc.vector.memset(ones, 1.0)
            w2ps = ps2p.tile([128, ((TW + 511) // 512) * 512], F32)
            w2bc = consts.tile([128, TW], FP16)
            xw_sb = consts.tile([128, T, 148], MM_DT)
            bw_sb = consts.tile([1, 2 * TW], FP16)
            ht_sb = consts.tile([128, T * 4, H5], FP16)
            m2_sb = consts.tile([128, T * 4, H5], FP16)
            gs_sb = consts.tile([128, T * 4], F32)
            y_sb = consts.tile([128, T * 4], MM_DT)

            # ---- DMA issue schedule ----
            # bias + W2 rows: tiny, first on the ACT HWDGE ring so its
            # completion (which gates every bias matmul) lands early
            nc.scalar.dma_start(out=bw_sb, in_=bw[:, :])
            # xw chunks alternate between the SP and ACT HWDGE rings so
            # descriptor issue never gates the stream and both rings start
            # with a big transfer
            for i in range(len(dchunks)):
                t0, t1 = dbounds[i], dbounds[i + 1]
                eng = nc.sync if i % 2 == 0 else nc.scalar
                eng.dma_start(out=xw_sb[:, t0:t1, :], in_=xw[:, t0:t1, :])

            # static [W2 | b2] broadcast: x-independent rank-1 matmuls into
            # a 3-bank f32 PSUM region, then one copy to fp16 SBUF for the
            # 16-bit DVE post-chain
            for k in range((TW + 511) // 512):
                n = min(512, TW - k * 512)
                nc.tensor.matmul(
                    out=w2ps[:, k * 512:k * 512 + n], lhsT=ones,
                    rhs=bw_sb[:, TW + k * 512:TW + k * 512 + n],
                    start=True, stop=True,
                )
            nc.vector.tensor_copy(w2bc, w2ps[:, :TW])

            # ---- compute: per-chunk matmuls + tanh, post-chain per batch ----
            bi = 0
            for ci in range(C):
                t0, t1 = bounds[ci], bounds[ci + 1]
                mt = t1 - t0
                g = mt * 4
                ps1 = ps1p.tile([128, g, H5], F32)
                for tt in range(mt):
                    nc.tensor.matmul(
                        out=ps1[:, tt * 4:(tt + 1) * 4, :],
                        lhsT=xw_sb[:, t0 + tt, :128],
                        rhs=xw_sb[:, t0 + tt, 128:],
                        # start=True resets has_written for the whole PSUM
                        # bank, so only the first matmul per bank may set it
                        start=(tt == 0), stop=False,
                    )
                # += b1 broadcast (rank-1: ones-column x bias row)
                nc.tensor.matmul(
                    out=ps1[:, :, :],
                    lhsT=ones,
                    rhs=bw_sb[:, t0 * 4 * H5:t1 * 4 * H5],
                    start=False, stop=True,
                )
                nc.scalar.activation(
                    out=ht_sb[:, t0 * 4:t1 * 4, :], in_=ps1,
                    func=mybir.ActivationFunctionType.Tanh)

                if bi < len(batches) and t1 == batches[bi][1]:
                    ga, gb = batches[bi][0] * 4, batches[bi][1] * 4
                    nc.vector.tensor_mul(
                        m2_sb[:, ga:gb, :], ht_sb[:, ga:gb, :],
                        w2bc[:, ga * H5:gb * H5].rearrange(
                            "p (g h) -> p g h", h=H5))
                    nc.vector.tensor_reduce(
                        out=gs_sb[:, ga:gb], in_=m2_sb[:, ga:gb, :],
                        axis=mybir.AxisListType.X, op=mybir.AluOpType.add)
                    nc.scalar.activation(
                        out=y_sb[:, ga:gb], in_=gs_sb[:, ga:gb],
                        func=mybir.ActivationFunctionType.Sigmoid)
                    nc.sync.dma_start(out=y[:, ga:gb], in_=y_sb[:, ga:gb])
                    bi += 1

    if split_waits:
        _split_excess_waits(nc)
    return nc


# ------------------------------------------------------------------- driver
def _run(inputs, trace=False):
    T, in_maps, scatter = _pack(**inputs)
    nc = _build(T)
    res = run_bass_kernel_spmd(
        nc, in_maps, core_ids=list(range(N_CORES)), trace=trace)
    out = np.zeros((B, 1), np.float32)
    for c in range(N_CORES):
        ydev = np.asarray(res.results[c]["y"], dtype=np.float32)  # (128, T*4)
        ypack = np.ascontiguousarray(ydev.T).ravel()  # packed slot order
        idx, valid = scatter[c]
        out[idx[valid], 0] = ypack[valid]
    return out, res


def kernel(**inputs):
    out, _ = _run(inputs, trace=False)
    return out
